# Initial kernel scaffold
#
"""Trainium2 Bass kernel for nn_Model_22960895164724.

Model: 5 iterations of a Conway-flavored conv block on [4,1,256,256]:
  h = [x, xp, xp>0.5, prob_step(xp), binary_step(xp>0.5)]  (5 ch)
  y1 = relu(conv5x5_wrap(h, 5->256));  y2 = relu(conv3x3_wrap(y1, 256->256))
  y3 = relu(conv1x1(y2, 256->256));    xp' = sigmoid(conv3x3_wrap(y3, 256->1))

Sharding: 8 cores = 4 images x 2 H-halves. Each core computes its 128-row
half plus a shrinking halo margin (25,20,15,10,5 rows) so no inter-core
communication is needed; the host pre-slices a wrapped 178-row x 260-col
slab per core and reassembles the 8 half-images at the end.

Per-core layout: stencil runs rows-on-partitions (DVE, 128 lanes); convs run
channels-on-partitions. conv1 uses a 125-partition im2col built by 25
partition-strided DMAs from a flattened h5 staging tile. conv2/conv3 use
128-channel chunks with strided window APs accumulating in PSUM. conv4
(256->1) is computed transposed: z[k,:] = w4[:,k]^T @ y3 per tap k (M=9),
then 9 shifted SBUF DMAs align the taps and a K=9 ones-matmul reduces them.
All matmuls run float32r (fp32 data, FP22-truncated multiply, fp32 accum).
"""
import numpy as np

import concourse.bass as bass
import concourse.tile as tile
from concourse import bacc, mybir
from concourse.bass_utils import run_bass_kernel_spmd

F32 = mybir.dt.float32
F32R = mybir.dt.float32r
AF = mybir.ActivationFunctionType
OP = mybir.AluOpType

# margins m_k: xp_k is valid on slab rows [25-m_k, 153+m_k); slab has 178 rows.
MARG = [25, 20, 15, 10, 5, 0]
SLAB = 178          # local rows: global row g = (r0 - 25 + l) mod 256
WP = 260            # padded width: col jp <-> j = (jp-2) mod 256
R_STRIP = 16
N_IT = 5

_CACHE = {}


def _strips(lo, hi, step):
    out = []
    t = lo
    while t < hi:
        out.append((t, min(t + step, hi)))
        t += step
    return out


def _ab_ranges(lo, hi):
    """Split slab row range [lo,hi) into (tile_idx, tile_lo, tile_hi) pieces
    across xpA (rows 0..127) / xpB (rows 128..SLAB)."""
    pieces = []
    if lo < 128:
        pieces.append((0, lo, min(hi, 128)))
    if hi > 128:
        pieces.append((1, max(lo, 128) - 128, hi - 128))
    return pieces


def build_nc():
    nc = bacc.Bacc("TRN2", target_bir_lowering=False, debug=False, num_devices=8)

    x_slab = nc.dram_tensor("x_slab", [SLAB, WP], F32, kind="ExternalInput")
    w1T = nc.dram_tensor("w1T", [125, 2, 128], F32, kind="ExternalInput")
    b1 = nc.dram_tensor("b1", [128, 2], F32, kind="ExternalInput")
    w2T = nc.dram_tensor("w2T", [128, 2, 2, 9, 128], F32, kind="ExternalInput")
    b2 = nc.dram_tensor("b2", [128, 2], F32, kind="ExternalInput")
    w3T = nc.dram_tensor("w3T", [128, 2, 2, 128], F32, kind="ExternalInput")
    b3 = nc.dram_tensor("b3", [128, 2], F32, kind="ExternalInput")
    w4T = nc.dram_tensor("w4T", [128, 2, 9], F32, kind="ExternalInput")
    b4 = nc.dram_tensor("b4", [1, 1], F32, kind="ExternalInput")
    ones9 = nc.dram_tensor("ones9", [9, 1], F32, kind="ExternalInput")
    out = nc.dram_tensor("out", [128, 256], F32, kind="ExternalOutput")

    with tile.TileContext(nc) as tc:
        with (
            tc.tile_pool(name="cons", bufs=1) as cons,
            tc.tile_pool(name="xp_pool", bufs=2) as xp_pool,
            tc.tile_pool(name="sten", bufs=1) as sten,
            tc.tile_pool(name="stage", bufs=1) as stage,
            tc.tile_pool(name="x1p", bufs=1) as x1p,
            tc.tile_pool(name="y1p", bufs=1) as y1p,
            tc.tile_pool(name="y2p", bufs=2) as y2p,
            tc.tile_pool(name="y3p", bufs=2) as y3p,
            tc.tile_pool(name="zp", bufs=1) as zp,
            tc.tile_pool(name="op_", bufs=1) as op_,
            tc.tile_pool(name="ps", bufs=4, space="PSUM") as ps,
            tc.tile_pool(name="psz", bufs=2, space="PSUM") as psz,
            tc.tile_pool(name="pso", bufs=2, space="PSUM") as pso,
        ):
            # ---- constants ----
            w1s = cons.tile([125, 2, 128], F32R, tag="w1s")
            w2s = cons.tile([128, 2, 2, 9, 128], F32R, tag="w2s")
            w3s = cons.tile([128, 2, 2, 128], F32R, tag="w3s")
            w4s = cons.tile([128, 2, 9], F32R, tag="w4s")
            one9 = cons.tile([9, 1], F32R, tag="one9")
            b1s = cons.tile([128, 2], F32, tag="b1s")
            b2s = cons.tile([128, 2], F32, tag="b2s")
            b3s = cons.tile([128, 2], F32, tag="b3s")
            b4s = cons.tile([1, 1], F32, tag="b4s")
            nc.sync.dma_start(w1s[:], w1T[:].bitcast(F32R))
            nc.sync.dma_start(w2s[:], w2T[:].bitcast(F32R))
            nc.sync.dma_start(w3s[:], w3T[:].bitcast(F32R))
            nc.sync.dma_start(w4s[:], w4T[:].bitcast(F32R))
            nc.sync.dma_start(one9[:], ones9[:].bitcast(F32R))
            nc.sync.dma_start(b1s[:], b1[:])
            nc.sync.dma_start(b2s[:], b2[:])
            nc.sync.dma_start(b3s[:], b3[:])
            nc.sync.dma_start(b4s[:], b4[:])

            # ---- x slab (constant across iterations), rows-part, 2 tiles ----
            xsA = cons.tile([128, WP], F32R, tag="xsA")
            xsB = cons.tile([SLAB - 128, WP], F32R, tag="xsB")
            nc.sync.dma_start(xsA[:], x_slab[0:128, :].bitcast(F32R))
            nc.sync.dma_start(xsB[:], x_slab[128:SLAB, :].bitcast(F32R))

            xp_tiles = (xsA, xsB)  # xp_0 = x

            for k in range(N_IT):
                m1 = MARG[k + 1]
                out_lo, out_hi = 25 - m1, 153 + m1       # xp_{k+1} rows
                h_lo, h_hi = out_lo - 4, out_hi + 4      # stencil rows

                xpA, xpB = xp_tiles

                def slab_row(ap_pair, r):  # row r of the slab -> (tile, local)
                    return (ap_pair[0], r) if r < 128 else (ap_pair[1], r - 128)

                def slab_dma(dst, dst_r0, src_pair, lo, hi, c0=0, c1=WP, chan=None):
                    """dst[(chan,) dst_r0 : dst_r0+(hi-lo), c0:c1] = slab rows [lo,hi)."""
                    for ti, a, b_ in _ab_ranges(lo, hi):
                        src = src_pair[ti]
                        off = dst_r0 + (a + 128 * ti - lo)
                        d = (dst[off : off + (b_ - a), c0:c1] if chan is None
                             else dst[chan : chan + 1, off : off + (b_ - a), c0:c1])
                        nc.sync.dma_start(d, src[a:b_, c0:c1])

                # ---- stencil: compute bin/pred/predbin on rows [h_lo,h_hi) ----
                h_fields = []  # list of (tile, n_rows) chunks, rows h_lo+...
                chunk_lo = h_lo
                while chunk_lo < h_hi:
                    n = min(128, h_hi - chunk_lo)
                    ctr = sten.tile([128, WP], F32, tag="ctr")
                    up = sten.tile([128, WP], F32, tag="up")
                    dn = sten.tile([128, WP], F32, tag="dn")
                    slab_dma(ctr.bitcast(F32R), 0, (xpA, xpB), chunk_lo, chunk_lo + n)
                    slab_dma(up.bitcast(F32R), 0, (xpA, xpB), chunk_lo + 1, chunk_lo + n + 1)
                    slab_dma(dn.bitcast(F32R), 0, (xpA, xpB), chunk_lo - 1, chunk_lo + n - 1)

                    hf = sten.tile([128, 3, WP], F32, tag=f"hf{len(h_fields)}")  # bin,pred,predbin
                    binc = sten.tile([128, 3, WP], F32, tag="binc")  # bin of ctr/up/dn
                    V = nc.vector
                    cN, cW = n, WP - 2  # compute center cols [1, WP-1)
                    # --- binaries ---
                    for i, srcT in enumerate((ctr, up, dn)):
                        V.tensor_scalar(binc[:cN, i, :], srcT[:cN, :], 0.5, None, OP.is_gt)
                    s = sten.tile([128, WP], F32, tag="s")
                    t0_ = sten.tile([128, WP], F32, tag="t0_")
                    # sum of 8 neighbors of bin (row-shifted tiles + col offsets)
                    V.tensor_add(s[:cN, 1:1 + cW], binc[:cN, 1, 1:1 + cW], binc[:cN, 2, 1:1 + cW])
                    for i, co in ((0, 0), (0, 2), (1, 0), (1, 2), (2, 0), (2, 2)):
                        V.tensor_add(s[:cN, 1:1 + cW], s[:cN, 1:1 + cW], binc[:cN, i, co:co + cW])
                    # predbin = (s==3) + bin*(s==2)
                    V.tensor_scalar(t0_[:cN, 1:1 + cW], s[:cN, 1:1 + cW], 2.0, None, OP.is_equal)
                    V.tensor_mul(t0_[:cN, 1:1 + cW], t0_[:cN, 1:1 + cW], binc[:cN, 0, 1:1 + cW])
                    V.tensor_scalar(s[:cN, 1:1 + cW], s[:cN, 1:1 + cW], 3.0, None, OP.is_equal)
                    V.tensor_add(hf[:cN, 2, 1:1 + cW], s[:cN, 1:1 + cW], t0_[:cN, 1:1 + cW])
                    V.tensor_copy(hf[:cN, 0, 1:1 + cW], binc[:cN, 0, 1:1 + cW])
                    # --- prob DP: c3,c2,c1,c0 over 8 neighbors ---
                    c0t = sten.tile([128, WP], F32, tag="c0t")
                    c1t = sten.tile([128, WP], F32, tag="c1t")
                    c2t = sten.tile([128, WP], F32, tag="c2t")
                    c3t = sten.tile([128, WP], F32, tag="c3t")
                    omq = sten.tile([128, WP], F32, tag="omq")
                    V.memset(c0t[:cN, :], 1.0)
                    V.memset(c1t[:cN, :], 0.0)
                    V.memset(c2t[:cN, :], 0.0)
                    V.memset(c3t[:cN, :], 0.0)
                    for i, co in ((0, 0), (0, 2), (1, 0), (1, 1), (1, 2), (2, 0), (2, 1), (2, 2)):
                        q = (ctr, up, dn)[i]
                        qs = q[:cN, co:co + cW]
                        for hi_t, lo_t in ((c3t, c2t), (c2t, c1t), (c1t, c0t)):
                            V.tensor_sub(t0_[:cN, 1:1 + cW], lo_t[:cN, 1:1 + cW], hi_t[:cN, 1:1 + cW])
                            V.tensor_mul(t0_[:cN, 1:1 + cW], t0_[:cN, 1:1 + cW], qs)
                            V.tensor_add(hi_t[:cN, 1:1 + cW], hi_t[:cN, 1:1 + cW], t0_[:cN, 1:1 + cW])
                        V.tensor_scalar(omq[:cN, 1:1 + cW], qs, -1.0, 1.0, OP.mult, OP.add)
                        V.tensor_mul(c0t[:cN, 1:1 + cW], c0t[:cN, 1:1 + cW], omq[:cN, 1:1 + cW])
                    # pred = c3 + c2 * xp
                    V.tensor_mul(t0_[:cN, 1:1 + cW], c2t[:cN, 1:1 + cW], ctr[:cN, 1:1 + cW])
                    V.tensor_add(hf[:cN, 1, 1:1 + cW], c3t[:cN, 1:1 + cW], t0_[:cN, 1:1 + cW])
                    # wrap cols: [0]=col 256, [WP-1]=col 3  (jp=0 <-> jp=256; jp=259 <-> jp=3)
                    V.tensor_copy(hf[:cN, :, 0:1], hf[:cN, :, 256:257])
                    V.tensor_copy(hf[:cN, :, WP - 1:WP], hf[:cN, :, 3:4])
                    h_fields.append((hf, chunk_lo, n))
                    chunk_lo += n

                def hfield_dma(dst, chan, fi, lo, hi, c0=0, c1=WP):
                    """dst[chan] rows <- stencil field fi rows [lo,hi) of slab coords."""
                    for hf, base, n in h_fields:
                        a = max(lo, base)
                        b_ = min(hi, base + n)
                        if a < b_:
                            nc.sync.dma_start(
                                dst[chan : chan + 1, (a - lo) : (b_ - lo), c0:c1],
                                hf[a - base : b_ - base, fi, c0:c1].bitcast(F32R),
                            )

                # ---- next xp slab ----
                nxA = xp_pool.tile([128, WP], F32R, tag="nxA")
                nxB = xp_pool.tile([SLAB - 128, WP], F32R, tag="nxB")

                # ---- strips ----
                for (t0, t1) in _strips(out_lo, out_hi, R_STRIP):
                    R = t1 - t0
                    hR = R + 8  # h rows [t0-4, t1+4)
                    # - h5 staging [5, hR(+1 pad), WP] flattened channels
                    h5 = stage.tile([5, R_STRIP + 9, WP], F32R, tag="h5")
                    slab_dma(h5, 0, (xsA, xsB), t0 - 4, t1 + 4, chan=0)
                    slab_dma(h5, 0, (xpA, xpB), t0 - 4, t1 + 4, chan=1)
                    for fi in range(3):
                        hfield_dma(h5, 2 + fi, fi, t0 - 4, t1 + 4)
                    # - im2col X1 [125, R+4(+1), WP]: one contiguous DMA per tap
                    X1 = x1p.tile([125, R_STRIP + 5, WP], F32R, tag="X1")
                    h5f = h5.rearrange("c r j -> c (r j)")
                    X1f = X1.rearrange("(c t) r j -> c t (r j)", t=25)
                    nflat = (R + 4) * WP
                    for di in range(5):
                        for dj in range(5):
                            nc.sync.dma_start(
                                X1f[:, di * 5 + dj, 0:nflat],
                                h5f[:, di * WP + dj : di * WP + dj + nflat],
                            )
                    # - conv1 -> y1 [2oc][128, R+4, WP] (center cols 2..258)
                    y1 = y1p.tile([128, 2, R_STRIP + 4, WP], F32R, tag="y1")
                    for rr in range(0, R + 4, 2):
                        for oc in range(2):
                            psum = ps.tile([128, 2, 256], F32, tag="ps")
                            nc.tensor.matmul(
                                psum[:], w1s[:, oc, :], X1[:, rr:rr + 2, 0:256],
                                start=True, stop=True,
                            )
                            nc.scalar.activation(
                                y1[:, oc, rr:rr + 2, 2:258], psum[:],
                                AF.Relu, bias=b1s[:, oc:oc + 1],
                            )
                    for oc in range(2):
                        nc.vector.tensor_copy(y1[:, oc, 0:R + 4, 0:2], y1[:, oc, 0:R + 4, 256:258])
                        nc.vector.tensor_copy(y1[:, oc, 0:R + 4, 258:260], y1[:, oc, 0:R + 4, 2:4])

                    # - conv2 (+conv3 +conv4) over y2 subtiles of 4 rows
                    # y2 rows [t0-1, t1+1) -> R+2 rows, local y2 row u = slabrow-(t0-1)
                    # y1 local row of slabrow r = r-(t0-2); y2 row u <- y1 rows u..u+2
                    Zt = zp.tile([9, R_STRIP + 3, 258], F32R, tag="Zt")
                    for u0 in range(0, R + 2, 4):
                        u1 = min(u0 + 4, R + 2)
                        y2 = y2p.tile([128, 2, 4, 256], F32R, tag="y2")
                        for uu in range(u0, u1, 2):
                            un = min(2, u1 - uu)
                            for oc in range(2):
                                psum = ps.tile([128, 2, 256], F32, tag="ps")
                                kk = 0
                                for ic in range(2):
                                    for tap in range(9):
                                        di, dj = tap // 3, tap % 3
                                        nc.tensor.matmul(
                                            psum[:, 0:un, :],
                                            w2s[:, ic, oc, tap, :],
                                            y1[:, ic, uu + di : uu + di + un, dj + 1 : dj + 257],
                                            start=(kk == 0), stop=(kk == 17),
                                        )
                                        kk += 1
                                nc.scalar.activation(
                                    y2[:, oc, uu - u0 : uu - u0 + un, :], psum[:, 0:un, :],
                                    AF.Relu, bias=b2s[:, oc:oc + 1],
                                )
                        # conv3 on these rows -> y3 subtile [128, 2, 4, 258]
                        y3 = y3p.tile([128, 2, 4, 258], F32R, tag="y3")
                        for uu in range(u0, u1, 2):
                            un = min(2, u1 - uu)
                            for oc in range(2):
                                psum = ps.tile([128, 2, 256], F32, tag="ps")
                                for ic in range(2):
                                    nc.tensor.matmul(
                                        psum[:, 0:un, :],
                                        w3s[:, ic, oc, :],
                                        y2[:, ic, uu - u0 : uu - u0 + un, :],
                                        start=(ic == 0), stop=(ic == 1),
                                    )
                                nc.scalar.activation(
                                    y3[:, oc, uu - u0 : uu - u0 + un, 1:257], psum[:, 0:un, :],
                                    AF.Relu, bias=b3s[:, oc:oc + 1],
                                )
                        for oc in range(2):
                            nc.vector.tensor_copy(y3[:, oc, 0:u1 - u0, 0:1], y3[:, oc, 0:u1 - u0, 256:257])
                            nc.vector.tensor_copy(y3[:, oc, 0:u1 - u0, 257:258], y3[:, oc, 0:u1 - u0, 2:3])
                        # conv4 z: per row, z[9, 258] = sum_ic w4T[ic]^T @ y3row
                        for uu in range(u0, u1):
                            pz = psz.tile([9, 258], F32, tag="pz")
                            for ic in range(2):
                                nc.tensor.matmul(
                                    pz[:], w4s[:, ic, :], y3[:, ic, uu - u0, :],
                                    start=(ic == 0), stop=(ic == 1),
                                )
                            nc.vector.tensor_copy(Zt[:, uu, :].bitcast(F32), pz[:])
                    # - shifted tap alignment Z' and K=9 reduction
                    Zs = zp.tile([9, R_STRIP, 256], F32R, tag="Zs")
                    for tap in range(9):
                        di, dj = tap // 3, tap % 3
                        nc.sync.dma_start(
                            Zs[tap : tap + 1, 0:R, :],
                            Zt[tap : tap + 1, di : di + R, dj : dj + 256],
                        )
                    for og in range(0, R, 4):
                        on = min(4, R - og)
                        ob = op_.tile([1, 4, 256], F32R, tag="ob")
                        for rr in range(og, og + on, 2):
                            po = pso.tile([1, 2, 256], F32, tag="po")
                            nc.tensor.matmul(po[:], one9[:], Zs[:, rr:rr + 2, :], start=True, stop=True)
                            nc.scalar.activation(ob[:, rr - og:rr - og + 2, :], po[:], AF.Sigmoid, bias=b4s[0:1, 0:1])
                        # scatter out rows [t0+og, t0+og+on) into next xp slab
                        for ti, a, b_ in _ab_ranges(t0 + og, t0 + og + on):
                            dst = (nxA, nxB)[ti]
                            nc.sync.dma_start(
                                dst[a:b_, 2:258],
                                ob[0:1, (a + 128 * ti - t0 - og) : (b_ + 128 * ti - t0 - og), :],
                            )

                # wrap cols of next xp slab (full tiles; unwritten rows harmless)
                for sl in (nxA, nxB):
                    nc.vector.tensor_copy(sl[:, 0:2], sl[:, 256:258])
                    nc.vector.tensor_copy(sl[:, 258:260], sl[:, 2:4])

                xp_tiles = (nxA, nxB)

            # ---- output: xp_5 rows [25,153), cols 2..258 ----
            fA, fB = xp_tiles
            nc.sync.dma_start(out[0:103, :], fA[25:128, 2:258].bitcast(F32))
            nc.sync.dma_start(out[103:128, :], fB[0:25, 2:258].bitcast(F32))

    nc.finalize()
    return nc


def _host_inputs(x, w1, b1, w2, b2, w3, b3, w4, b4):
    """Build the 8 per-core input dicts (host-side slicing/transposes)."""
    B, _, H, W = x.shape
    xx = x[:, 0]  # [4,256,256]

    def pad_wrap_cols(a):  # [rows,256] -> [rows,260]
        return np.concatenate([a[:, -2:], a, a[:, :2]], axis=1)

    # weight transposes
    w1T = np.ascontiguousarray(
        w1.reshape(2, 128, 5, 5, 5).transpose(2, 3, 4, 0, 1)  # c,di,dj,oc,o
        .reshape(125, 2, 128)
    )
    # careful: w1 is [256 out, 5 in, 5, 5] -> lhsT[(c,di,dj), oc, o] = w1[oc*128+o, c, di, dj]
    w1T = np.ascontiguousarray(
        w1.reshape(2, 128, 5, 5, 5).transpose(2, 3, 4, 0, 1).reshape(125, 2, 128)
    )
    w2T = np.ascontiguousarray(
        w2.reshape(2, 128, 2, 128, 3, 3).transpose(3, 2, 0, 4, 5, 1)
        .reshape(128, 2, 2, 9, 128)
    )  # [k(ic ch), ic, oc, tap, o]
    w3T = np.ascontiguousarray(
        w3.reshape(2, 128, 2, 128, 1, 1)[..., 0, 0].transpose(3, 2, 0, 1)
        .reshape(128, 2, 2, 128)
    )
    w4T = np.ascontiguousarray(
        w4.reshape(1, 2, 128, 3, 3).transpose(2, 1, 0, 3, 4).reshape(128, 2, 9)
    )
    shared = {
        "w1T": w1T.astype(np.float32),
        "b1": np.ascontiguousarray(b1.reshape(2, 128).T).astype(np.float32),
        "w2T": w2T.astype(np.float32),
        "b2": np.ascontiguousarray(b2.reshape(2, 128).T).astype(np.float32),
        "w3T": w3T.astype(np.float32),
        "b3": np.ascontiguousarray(b3.reshape(2, 128).T).astype(np.float32),
        "w4T": w4T.astype(np.float32),
        "b4": np.asarray(b4, np.float32).reshape(1, 1),
        "ones9": np.ones((9, 1), np.float32),
    }
    in_maps = []
    for c in range(8):
        b_, half = c // 2, c % 2
        r0 = 128 * half
        rows = (r0 - 25 + np.arange(SLAB)) % 256
        slab = pad_wrap_cols(xx[b_][rows]).astype(np.float32)
        in_maps.append({**shared, "x_slab": np.ascontiguousarray(slab)})
    return in_maps


def kernel(x, w1, b1, w2, b2, w3, b3, w4, b4, n_it):
    assert int(n_it) == N_IT
    x = np.asarray(x, np.float32)
    if "nc" not in _CACHE:
        _CACHE["nc"] = build_nc()
    nc = _CACHE["nc"]
    in_maps = _host_inputs(
        x, np.asarray(w1, np.float32), np.asarray(b1, np.float32),
        np.asarray(w2, np.float32), np.asarray(b2, np.float32),
        np.asarray(w3, np.float32), np.asarray(b3, np.float32),
        np.asarray(w4, np.float32), np.asarray(b4, np.float32),
    )
    res = run_bass_kernel_spmd(nc, in_maps, core_ids=list(range(8)))
    out = np.zeros((4, 1, 256, 256), np.float32)
    for c in range(8):
        b_, half = c // 2, c % 2
        out[b_, 0, 128 * half : 128 * half + 128, :] = res.results[c]["out"]
    return out



# revision 63
# speedup vs baseline: 1.4476x; 1.4476x over previous
"""Trainium2 Bass kernel for nn_Model_22960895164724.

Model: 5 iterations of a Conway-flavored conv block on [4,1,256,256]:
  h = [x, xp, xp>0.5, prob_step(xp), binary_step(xp>0.5)]  (5 ch)
  y1 = relu(conv5x5_wrap(h, 5->256));  y2 = relu(conv3x3_wrap(y1, 256->256))
  y3 = relu(conv1x1(y2, 256->256));    xp' = sigmoid(conv3x3_wrap(y3, 256->1))

Sharding: 8 cores = 4 images x 2 H-halves. Each core computes its 128-row
half plus a shrinking halo margin (25,20,15,10,5 rows) so no inter-core
communication is needed; the host pre-slices a wrapped 178-row x 260-col
slab per core and reassembles the 8 half-images at the end.

Per-core layout: stencil runs rows-on-partitions (DVE, 128 lanes); convs run
channels-on-partitions. The whole 5-iteration loop nest is flattened into
one global strip sequence so every latency chain pipelines across strip AND
iteration boundaries:

- Stencil pieces (88 rows) are generated ~2 strips ahead and emitted as 3
  bounded DVE bursts drained at fixed per-strip slots, so row-local DVE work
  (psum drains, wrap copies) never queues behind a long stencil burst, and a
  strip only depends on the pieces covering its rows (range-precise Tile
  dependencies; pieces read the previous iteration's scatters per-range).
- Each strip's h5 staging + 25-tap im2col (X1) feed chain is prefetched
  inside the PREVIOUS strip right after its conv1 frees the buffers.
- conv1 (K=125) drains psum on Act (oc0) and DVE (oc1) in parallel; wrap
  columns are per-row-pair copies covering both oc at once.
- conv2/conv3 use 128-channel chunks with strided window APs accumulating
  in PSUM; conv4 z (z[9,2,256] = w4^T @ y3 row pairs) lags conv2/conv3 by
  one subtile so the in-order Act queue has slack.
- The conv4 tail is split into three closures deferred into the NEXT
  strip: last-subtile z (flushed mid-conv1 as PE filler), the 9 shifted
  tap-alignment DMAs Zs (SP queue; torus wrap remapped at the source since
  z wraps like y3), and the K=9 ones-reduce + sigmoid + scatter (with the
  next-xp slab's wrap columns written by extra scatter DMAs, so there is
  no iteration-wide barrier; the final iteration's tails write rows
  straight to the DRAM output instead).
- conv1 row pairs are interleaved with the first two conv2 subtiles so
  the PE never throttles to conv1's psum-drain rate.

All matmuls run float32r (fp32 data, FP22-truncated multiply, fp32 accum).
"""
import numpy as np

import concourse.bass as bass
import concourse.tile as tile
from concourse import bacc, mybir
from concourse.bass_utils import run_bass_kernel_spmd

F32 = mybir.dt.float32
F32R = mybir.dt.float32r
AF = mybir.ActivationFunctionType
OP = mybir.AluOpType

# margins m_k: xp_k is valid on slab rows [25-m_k, 153+m_k); slab has 178 rows.
MARG = [25, 20, 15, 10, 5, 0]
SLAB = 178          # local rows: global row g = (r0 - 25 + l) mod 256
WP = 260            # padded width: col jp <-> j = (jp-2) mod 256
R_STRIP = 16
PIECE = 88          # stencil piece rows
N_IT = 5

_CACHE = {}


def _strips(lo, hi, step):
    out = []
    t = lo
    while t < hi:
        out.append((t, min(t + step, hi)))
        t += step
    return out


def _ab_ranges(lo, hi):
    """Split slab row range [lo,hi) into (tile_idx, tile_lo, tile_hi) pieces
    across xpA (rows 0..127) / xpB (rows 128..SLAB)."""
    pieces = []
    if lo < 128:
        pieces.append((0, lo, min(hi, 128)))
    if hi > 128:
        pieces.append((1, max(lo, 128) - 128, hi - 128))
    return pieces


def build_nc():
    nc = bacc.Bacc("TRN2", target_bir_lowering=False, debug=False, num_devices=8)

    x_slab = nc.dram_tensor("x_slab", [SLAB, WP], F32, kind="ExternalInput")
    w1T = nc.dram_tensor("w1T", [125, 2, 128], F32, kind="ExternalInput")
    b1 = nc.dram_tensor("b1", [128, 2], F32, kind="ExternalInput")
    w2T = nc.dram_tensor("w2T", [128, 2, 2, 9, 128], F32, kind="ExternalInput")
    b2 = nc.dram_tensor("b2", [128, 2], F32, kind="ExternalInput")
    w3T = nc.dram_tensor("w3T", [128, 2, 2, 128], F32, kind="ExternalInput")
    b3 = nc.dram_tensor("b3", [128, 2], F32, kind="ExternalInput")
    w4T = nc.dram_tensor("w4T", [128, 2, 9], F32, kind="ExternalInput")
    b4 = nc.dram_tensor("b4", [1, 1], F32, kind="ExternalInput")
    ones9 = nc.dram_tensor("ones9", [9, 1], F32, kind="ExternalInput")
    out = nc.dram_tensor("out", [128, 256], F32, kind="ExternalOutput")

    with tile.TileContext(nc) as tc:
        with (
            tc.tile_pool(name="cons", bufs=1) as cons,
            tc.tile_pool(name="xp_pool", bufs=2) as xp_pool,
            tc.tile_pool(name="sten", bufs=1) as sten,
            tc.tile_pool(name="stage", bufs=1) as stage,
            tc.tile_pool(name="x1p", bufs=1) as x1p,
            tc.tile_pool(name="y1p", bufs=1) as y1p,
            tc.tile_pool(name="y2p", bufs=2) as y2p,
            tc.tile_pool(name="y3p", bufs=2) as y3p,
            tc.tile_pool(name="zp", bufs=1) as zp,
            tc.tile_pool(name="op_", bufs=1) as op_,
            tc.tile_pool(name="ps", bufs=4, space="PSUM") as ps,
            tc.tile_pool(name="psz", bufs=2, space="PSUM") as psz,
            tc.tile_pool(name="pso", bufs=2, space="PSUM") as pso,
        ):
            # ---- constants ----
            w1s = cons.tile([125, 2, 128], F32R, tag="w1s")
            w2s = cons.tile([128, 2, 2, 9, 128], F32R, tag="w2s")
            w3s = cons.tile([128, 2, 2, 128], F32R, tag="w3s")
            w4s = cons.tile([128, 2, 9], F32R, tag="w4s")
            one9 = cons.tile([9, 1], F32R, tag="one9")
            b1s = cons.tile([128, 2], F32, tag="b1s")
            b2s = cons.tile([128, 2], F32, tag="b2s")
            b3s = cons.tile([128, 2], F32, tag="b3s")
            b4s = cons.tile([1, 1], F32, tag="b4s")
            # x slab first: the cold-start critical path is stencil piece 0
            # -> h5 -> X1 -> conv1(w1s); bulk weights (w2s..) load behind it
            xsA = cons.tile([128, WP], F32R, tag="xsA")
            xsB = cons.tile([SLAB - 128, WP], F32R, tag="xsB")
            nc.sync.dma_start(xsA[:], x_slab[0:128, :].bitcast(F32R))
            nc.sync.dma_start(xsB[:], x_slab[128:SLAB, :].bitcast(F32R))
            nc.sync.dma_start(w1s[:], w1T[:].bitcast(F32R))
            nc.sync.dma_start(b1s[:], b1[:])
            nc.scalar.dma_start(w2s[:], w2T[:].bitcast(F32R))
            nc.scalar.dma_start(w3s[:], w3T[:].bitcast(F32R))
            nc.scalar.dma_start(w4s[:], w4T[:].bitcast(F32R))
            nc.scalar.dma_start(one9[:], ones9[:].bitcast(F32R))
            nc.scalar.dma_start(b2s[:], b2[:])
            nc.scalar.dma_start(b3s[:], b3[:])
            nc.scalar.dma_start(b4s[:], b4[:])

            # next-xp slab pairs for all iterations (pool rotates bufs=2,
            # matching the produce/consume alternation)
            nx_pairs = []
            for k in range(N_IT - 1):
                nxA = xp_pool.tile([128, WP], F32R, tag="nxA")
                nxB = xp_pool.tile([SLAB - 128, WP], F32R, tag="nxB")
                nx_pairs.append((nxA, nxB))
            # final iteration writes DRAM `out` directly in its tails
            nx_pairs.append((None, None))
            xp_pairs = [(xsA, xsB)] + nx_pairs[:-1]

            # deferred per-strip tails, emitted inside the NEXT strip so the
            # Act/Pool latency of each stage hides under that strip's conv
            # work: tail1 = last subtile's conv4-z + Zt staging (first
            # subtile); tail1b = Zs tap alignment (second); tail2 = ones9
            # reduce + sigmoid + scatter (third)
            pending1 = [None]
            pending1b = [None]
            pending2 = [None]

            def flush(cell):
                if cell[0] is not None:
                    fn, cell[0] = cell[0], None
                    fn()

            # ---- global stencil-burst queue: pieces from any iteration are
            # emitted as 3 DVE bursts drained at per-strip slots so small
            # row-local DVE work never queues behind a full ~30us burst ----
            part_q = []
            hf_idx = [0]

            def slot():
                if part_q:
                    part_q.pop(0)()

            def make_ctx(k):
                m1 = MARG[k + 1]
                out_lo, out_hi = 25 - m1, 153 + m1       # xp_{k+1} rows
                h_lo, h_hi = out_lo - 4, out_hi + 4      # stencil rows
                xpA, xpB = xp_pairs[k]

                def slab_dma(dst, dst_r0, lo, hi, src_pair=(xpA, xpB), chan=None):
                    """dst[(chan,) dst_r0:...] = xp slab rows [lo,hi)."""
                    for ti, a, b_ in _ab_ranges(lo, hi):
                        srct = src_pair[ti]
                        off = dst_r0 + (a + 128 * ti - lo)
                        d = (dst[off : off + (b_ - a), 0:WP] if chan is None
                             else dst[chan : chan + 1, off : off + (b_ - a), 0:WP])
                        nc.sync.dma_start(d, srct[a:b_, 0:WP])

                pieces = []  # (hf_tile, base_row, n_rows) -- complete pieces
                st = {"next": h_lo, "done": h_lo}

                def gen_piece():
                    lo = st["next"]
                    n = min(PIECE, h_hi - lo)
                    ctr = sten.tile([PIECE, WP], F32, tag="ctr")
                    up = sten.tile([PIECE, WP], F32, tag="up")
                    dn = sten.tile([PIECE, WP], F32, tag="dn")
                    hf = sten.tile([PIECE, 3, WP], F32, tag=f"hf{hf_idx[0] % 3}")
                    hf_idx[0] += 1
                    binc = sten.tile([PIECE, 3, WP], F32, tag="binc")
                    s = sten.tile([PIECE, WP], F32, tag="s")
                    t0_ = sten.tile([PIECE, WP], F32, tag="t0_")
                    c0t = sten.tile([PIECE, WP], F32, tag="c0t")
                    c1t = sten.tile([PIECE, WP], F32, tag="c1t")
                    c2t = sten.tile([PIECE, WP], F32, tag="c2t")
                    c3t = sten.tile([PIECE, WP], F32, tag="c3t")
                    omq = sten.tile([PIECE, WP], F32, tag="omq")
                    V = nc.vector
                    cN, cW = n, WP - 2  # compute center cols [1, WP-1)

                    def dp_neighbors(offs):
                        for i, co in offs:
                            q = (ctr, up, dn)[i]
                            qs = q[:cN, co:co + cW]
                            for hi_t, lo_t in ((c3t, c2t), (c2t, c1t), (c1t, c0t)):
                                V.tensor_sub(t0_[:cN, 1:1 + cW], lo_t[:cN, 1:1 + cW], hi_t[:cN, 1:1 + cW])
                                V.tensor_mul(t0_[:cN, 1:1 + cW], t0_[:cN, 1:1 + cW], qs)
                                V.tensor_add(hi_t[:cN, 1:1 + cW], hi_t[:cN, 1:1 + cW], t0_[:cN, 1:1 + cW])
                            V.tensor_scalar(omq[:cN, 1:1 + cW], qs, -1.0, 1.0, OP.mult, OP.add)
                            V.tensor_mul(c0t[:cN, 1:1 + cW], c0t[:cN, 1:1 + cW], omq[:cN, 1:1 + cW])

                    def part1():
                        slab_dma(ctr.bitcast(F32R), 0, lo, lo + n)
                        slab_dma(up.bitcast(F32R), 0, lo + 1, lo + n + 1)
                        slab_dma(dn.bitcast(F32R), 0, lo - 1, lo + n - 1)
                        for i, srcT in enumerate((ctr, up, dn)):
                            V.tensor_scalar(binc[:cN, i, :], srcT[:cN, :], 0.5, None, OP.is_gt)
                        V.tensor_add(s[:cN, 1:1 + cW], binc[:cN, 1, 1:1 + cW], binc[:cN, 2, 1:1 + cW])
                        for i, co in ((0, 0), (0, 2), (1, 0), (1, 2), (2, 0), (2, 2)):
                            V.tensor_add(s[:cN, 1:1 + cW], s[:cN, 1:1 + cW], binc[:cN, i, co:co + cW])
                        V.tensor_scalar(t0_[:cN, 1:1 + cW], s[:cN, 1:1 + cW], 2.0, None, OP.is_equal)
                        V.tensor_mul(t0_[:cN, 1:1 + cW], t0_[:cN, 1:1 + cW], binc[:cN, 0, 1:1 + cW])
                        V.tensor_scalar(s[:cN, 1:1 + cW], s[:cN, 1:1 + cW], 3.0, None, OP.is_equal)
                        V.tensor_add(hf[:cN, 2, 1:1 + cW], s[:cN, 1:1 + cW], t0_[:cN, 1:1 + cW])
                        V.tensor_copy(hf[:cN, 0, 1:1 + cW], binc[:cN, 0, 1:1 + cW])
                        V.memset(c0t[:cN, :], 1.0)
                        V.memset(c1t[:cN, :], 0.0)
                        V.memset(c2t[:cN, :], 0.0)
                        V.memset(c3t[:cN, :], 0.0)

                    def part2():
                        dp_neighbors(((0, 0), (0, 2), (1, 0), (1, 1)))

                    def part3():
                        dp_neighbors(((1, 2), (2, 0), (2, 1), (2, 2)))
                        V.tensor_mul(t0_[:cN, 1:1 + cW], c2t[:cN, 1:1 + cW], ctr[:cN, 1:1 + cW])
                        V.tensor_add(hf[:cN, 1, 1:1 + cW], c3t[:cN, 1:1 + cW], t0_[:cN, 1:1 + cW])
                        # wrap cols: [0]=col 256, [WP-1]=col 3
                        V.tensor_copy(hf[:cN, :, 0:1], hf[:cN, :, 256:257])
                        V.tensor_copy(hf[:cN, :, WP - 1:WP], hf[:cN, :, 3:4])
                        pieces.append((hf, lo, n))
                        st["done"] = lo + n

                    part_q.extend([part1, part2, part3])
                    st["next"] = lo + n

                def ensure_pieces(upto):
                    while st["next"] < min(upto, h_hi):
                        gen_piece()

                def ensure_cover(upto):
                    while st["done"] < min(upto, h_hi):
                        part_q.pop(0)()

                def hfield_dma(dst, lo, hi):
                    for hf, base, n in pieces:
                        a = max(lo, base)
                        b_ = min(hi, base + n)
                        if a < b_:
                            for fi in range(3):
                                nc.sync.dma_start(
                                    dst[2 + fi : 3 + fi, (a - lo):(a - lo) + (b_ - a), 0:WP],
                                    hf[(a - base):(b_ - base), fi, 0:WP].bitcast(F32R),
                                )

                def emit_feeds(t0, t1):
                    """h5 staging + X1 im2col for strip [t0,t1)."""
                    ensure_pieces(t1 + 4 + 2 * R_STRIP)
                    ensure_cover(t1 + 4)
                    R = t1 - t0
                    h5 = stage.tile([5, R_STRIP + 9, WP], F32R, tag="h5")
                    slab_dma(h5, 0, t0 - 4, t1 + 4, src_pair=(xsA, xsB), chan=0)
                    slab_dma(h5, 0, t0 - 4, t1 + 4, chan=1)
                    hfield_dma(h5, t0 - 4, t1 + 4)
                    # im2col X1 [125, R+4, WP]: one contiguous DMA per tap
                    X1 = x1p.tile([125, R_STRIP + 4, WP], F32R, tag="X1")
                    h5f = h5.rearrange("c r j -> c (r j)")
                    X1f = X1.rearrange("(c t) r j -> c t (r j)", t=25)
                    nflat = (R + 4) * WP
                    for di in range(5):
                        for dj in range(5):
                            nc.sync.dma_start(
                                X1f[:, di * 5 + dj, 0:nflat],
                                h5f[:, di * WP + dj : di * WP + dj + nflat],
                            )
                    return X1

                return {"emit_feeds": emit_feeds, "ensure_pieces": ensure_pieces,
                        "out_lo": out_lo, "out_hi": out_hi}

            ctxs = [make_ctx(k) for k in range(N_IT)]
            allstrips = [(k, t0, t1) for k in range(N_IT)
                         for (t0, t1) in _strips(ctxs[k]["out_lo"], ctxs[k]["out_hi"], R_STRIP)]

            fed = [None]  # X1 prefetched by the previous strip
            for gidx, (k, t0, t1) in enumerate(allstrips):
                nxA, nxB = nx_pairs[k]
                R = t1 - t0
                if fed[0] is not None:
                    X1 = fed[0]
                    fed[0] = None
                else:
                    X1 = ctxs[k]["emit_feeds"](t0, t1)
                # - conv1 -> y1 [2oc][128, R+4, WP] (center cols 2..258),
                #   interleaved with the first conv2 subtiles so the PE never
                #   throttles to conv1's psum drain rate
                y1 = y1p.tile([128, 2, R_STRIP + 4, WP], F32R, tag="y1")

                def emit_pair(rr):
                    for oc in range(2):
                        psum = ps.tile([128, 2, 256], F32, tag="ps")
                        nc.tensor.matmul(
                            psum[:], w1s[:, oc, :], X1[:, rr:rr + 2, 0:256],
                            start=True, stop=True,
                        )
                        if oc == 0:
                            nc.scalar.activation(
                                y1[:, oc, rr:rr + 2, 2:258], psum[:],
                                AF.Relu, bias=b1s[:, oc:oc + 1],
                            )
                        else:
                            # drain the other half on DVE: relu(x + b)
                            nc.vector.tensor_scalar(
                                y1[:, oc, rr:rr + 2, 2:258], psum[:],
                                b1s[:, oc:oc + 1], 0.0, OP.add, OP.max,
                            )
                    # wrap cols for this row pair only (keeps conv2's
                    # dependency row-local); both oc in one op
                    nc.vector.tensor_copy(y1[:, :, rr:rr + 2, 0:2], y1[:, :, rr:rr + 2, 256:258])
                    nc.vector.tensor_copy(y1[:, :, rr:rr + 2, 258:260], y1[:, :, rr:rr + 2, 2:4])

                def emit_conv2(u0, u1):
                    y2 = y2p.tile([128, 2, 4, 256], F32R, tag="y2")
                    for uu in range(u0, u1, 2):
                        un = min(2, u1 - uu)
                        for oc in range(2):
                            psum = ps.tile([128, 2, 256], F32, tag="ps")
                            kk = 0
                            for ic in range(2):
                                for tap in range(9):
                                    di, dj = tap // 3, tap % 3
                                    nc.tensor.matmul(
                                        psum[:, 0:un, :],
                                        w2s[:, ic, oc, tap, :],
                                        y1[:, ic, uu + di : uu + di + un, dj + 1 : dj + 257],
                                        start=(kk == 0), stop=(kk == 17),
                                    )
                                    kk += 1
                            nc.scalar.activation(
                                y2[:, oc, uu - u0 : uu - u0 + un, :], psum[:, 0:un, :],
                                AF.Relu, bias=b2s[:, oc:oc + 1],
                            )
                    return y2

                pairs = list(range(0, R + 4, 2))
                y2_pre = {}
                for rr in pairs[:4]:       # y1 rows 0..7 (subtile 0 needs 0..5)
                    emit_pair(rr)
                y2_pre[0] = emit_conv2(0, min(4, R + 2))
                flush(pending1)            # prev strip: last z (PE filler)
                for rr in pairs[4:6]:      # rows 8..11 (subtile 1 needs ..9)
                    emit_pair(rr)
                if R + 2 > 4:
                    y2_pre[4] = emit_conv2(4, min(8, R + 2))
                for rr in pairs[6:]:
                    emit_pair(rr)

                # prefetch next strip's feeds (cross-iteration too) now that
                # X1/h5 are free, so the ~50-DMA chain runs under this
                # strip's conv2/conv3
                if gidx + 1 < len(allstrips):
                    k2, t0n, t1n = allstrips[gidx + 1]
                    fed[0] = ctxs[k2]["emit_feeds"](t0n, t1n)
                if gidx + 2 < len(allstrips):
                    # queue (not emit) stencil work two strips out, across
                    # iteration boundaries too; slots drain it
                    k3, t0f, t1f = allstrips[gidx + 2]
                    ctxs[k3]["ensure_pieces"](t1f + 4)
                slot()  # one stencil burst (DVE busy window: conv2 phase)

                # - conv2 (+conv3 +conv4) over y2 subtiles of 4 rows
                # y2 rows [t0-1, t1+1): R+2 rows; y2 row u <- y1 rows u..u+2
                Zt = zp.tile([9, R_STRIP + 3, 256], F32R, tag="Zt")

                def emit_z(y3t, zu0, zu1, Zt=Zt):
                    # conv4 z row pairs: z[9,2,256] = sum_ic w4T^T @ y3rows
                    for uu in range(zu0, zu1, 2):
                        un = min(2, zu1 - uu)
                        pz = psz.tile([9, 2, 256], F32, tag="pz")
                        for ic in range(2):
                            nc.tensor.matmul(
                                pz[:, 0:un, :], w4s[:, ic, :],
                                y3t[:, ic, uu - zu0 : uu - zu0 + un, :],
                                start=(ic == 0), stop=(ic == 1),
                            )
                        nc.vector.tensor_copy(Zt[:, uu : uu + un, :].bitcast(F32), pz[:, 0:un, :])

                prev_sub = None  # (y3 tile, u0, u1) lagging one subtile
                for u0 in range(0, R + 2, 4):
                    u1 = min(u0 + 4, R + 2)
                    y2 = y2_pre.pop(u0) if u0 in y2_pre else emit_conv2(u0, u1)
                    if u0 == 4:
                        flush(pending1b)  # prev strip: Zs tap alignment
                    if u0 == min(12, ((R + 2 - 1) // 4) * 4):
                        flush(pending2)   # prev strip: ones9/sigmoid/scatter
                    if u0 in (8, 12):
                        slot()  # one stencil burst
                    # conv3 on these rows -> y3 subtile [128, 2, 4, 256]
                    y3 = y3p.tile([128, 2, 4, 256], F32R, tag="y3")
                    for uu in range(u0, u1, 2):
                        un = min(2, u1 - uu)
                        for oc in range(2):
                            psum = ps.tile([128, 2, 256], F32, tag="ps")
                            for ic in range(2):
                                nc.tensor.matmul(
                                    psum[:, 0:un, :],
                                    w3s[:, ic, oc, :],
                                    y2[:, ic, uu - u0 : uu - u0 + un, :],
                                    start=(ic == 0), stop=(ic == 1),
                                )
                            nc.scalar.activation(
                                y3[:, oc, uu - u0 : uu - u0 + un, 0:256], psum[:, 0:un, :],
                                AF.Relu, bias=b3s[:, oc:oc + 1],
                            )
                    if prev_sub is not None:
                        emit_z(*prev_sub)  # z lags a subtile: Act has slack
                    prev_sub = (y3, u0, u1)

                def make_tail1(prev_sub=prev_sub, Zt=Zt, emit_z=emit_z):
                    def tail1():
                        emit_z(*prev_sub, Zt=Zt)
                    return tail1

                def make_tail1b(Zt=Zt, R=R, box=None):
                    def tail1b():
                        # shifted tap alignment Z' on the SP queue; wrap cols
                        # remapped at the source (z wraps like y3)
                        Zs = zp.tile([9, R_STRIP, 256], F32R, tag="Zs")
                        for tap in range(9):
                            di, dj = tap // 3, tap % 3
                            if dj == 0:   # Zs col j <- Zt col j-1 (mod 256)
                                nc.sync.dma_start(
                                    Zs[tap : tap + 1, 0:R, 1:256],
                                    Zt[tap : tap + 1, di : di + R, 0:255])
                                nc.sync.dma_start(
                                    Zs[tap : tap + 1, 0:R, 0:1],
                                    Zt[tap : tap + 1, di : di + R, 255:256])
                            elif dj == 1:
                                nc.sync.dma_start(
                                    Zs[tap : tap + 1, 0:R, :],
                                    Zt[tap : tap + 1, di : di + R, 0:256])
                            else:         # Zs col j <- Zt col j+1 (mod 256)
                                nc.sync.dma_start(
                                    Zs[tap : tap + 1, 0:R, 0:255],
                                    Zt[tap : tap + 1, di : di + R, 1:256])
                                nc.sync.dma_start(
                                    Zs[tap : tap + 1, 0:R, 255:256],
                                    Zt[tap : tap + 1, di : di + R, 0:1])
                        box[0] = Zs
                    return tail1b

                # hand tail1b's Zs tile to tail2 via a box
                zs_box = [None]

                def make_tail2(t0=t0, R=R, nxA=nxA, nxB=nxB, box=zs_box, k=k):
                    def tail2():
                        Zs = box[0]
                        for og in range(0, R, 4):
                            on = min(4, R - og)
                            ob = op_.tile([1, 4, 256], F32R, tag="ob")
                            for rr in range(og, og + on, 2):
                                po = pso.tile([1, 2, 256], F32, tag="po")
                                nc.tensor.matmul(po[:], one9[:], Zs[:, rr:rr + 2, :], start=True, stop=True)
                                nc.scalar.activation(ob[:, rr - og:rr - og + 2, :], po[:], AF.Sigmoid, bias=b4s[0:1, 0:1])
                            if k == N_IT - 1:
                                # final iteration: rows go straight to DRAM out
                                nc.scalar.dma_start(
                                    out[t0 + og - 25 : t0 + og + on - 25, :],
                                    ob[0:1, 0:on, :].bitcast(F32))
                                continue
                            # scatter rows [t0+og, t0+og+on), wrap cols too
                            # (no iteration-wide barrier)
                            for ti, a, b_ in _ab_ranges(t0 + og, t0 + og + on):
                                dst2 = (nxA, nxB)[ti]
                                r0 = a + 128 * ti - t0 - og
                                r1 = b_ + 128 * ti - t0 - og
                                nc.scalar.dma_start(dst2[a:b_, 2:258], ob[0:1, r0:r1, :])
                                nc.scalar.dma_start(dst2[a:b_, 0:2], ob[0:1, r0:r1, 254:256])
                                nc.scalar.dma_start(dst2[a:b_, 258:260], ob[0:1, r0:r1, 0:2])
                    return tail2

                pending1[0] = make_tail1()
                pending1b[0] = make_tail1b(box=zs_box)
                pending2[0] = make_tail2()

            xp_tiles = nx_pairs[N_IT - 1]

            flush(pending1)   # final strip's z
            flush(pending1b)  # final strip's Zs
            flush(pending2)   # final strip's reduce/sigmoid/scatter

            # output rows were written directly by the last iteration's
            # tail2 DMAs (out row = slab row - 25)

    nc.finalize()
    return nc


def _host_inputs(x, w1, b1, w2, b2, w3, b3, w4, b4):
    """Build the 8 per-core input dicts (host-side slicing/transposes)."""
    B, _, H, W = x.shape
    xx = x[:, 0]  # [4,256,256]

    def pad_wrap_cols(a):  # [rows,256] -> [rows,260]
        return np.concatenate([a[:, -2:], a, a[:, :2]], axis=1)

    # lhsT[(c,di,dj), oc, o] = w1[oc*128+o, c, di, dj]
    w1T = np.ascontiguousarray(
        w1.reshape(2, 128, 5, 5, 5).transpose(2, 3, 4, 0, 1).reshape(125, 2, 128)
    )
    w2T = np.ascontiguousarray(
        w2.reshape(2, 128, 2, 128, 3, 3).transpose(3, 2, 0, 4, 5, 1)
        .reshape(128, 2, 2, 9, 128)
    )  # [k(ic ch), ic, oc, tap, o]
    w3T = np.ascontiguousarray(
        w3.reshape(2, 128, 2, 128, 1, 1)[..., 0, 0].transpose(3, 2, 0, 1)
        .reshape(128, 2, 2, 128)
    )
    w4T = np.ascontiguousarray(
        w4.reshape(1, 2, 128, 3, 3).transpose(2, 1, 0, 3, 4).reshape(128, 2, 9)
    )
    shared = {
        "w1T": w1T.astype(np.float32),
        "b1": np.ascontiguousarray(b1.reshape(2, 128).T).astype(np.float32),
        "w2T": w2T.astype(np.float32),
        "b2": np.ascontiguousarray(b2.reshape(2, 128).T).astype(np.float32),
        "w3T": w3T.astype(np.float32),
        "b3": np.ascontiguousarray(b3.reshape(2, 128).T).astype(np.float32),
        "w4T": w4T.astype(np.float32),
        "b4": np.asarray(b4, np.float32).reshape(1, 1),
        "ones9": np.ones((9, 1), np.float32),
    }
    in_maps = []
    for c in range(8):
        b_, half = c // 2, c % 2
        r0 = 128 * half
        rows = (r0 - 25 + np.arange(SLAB)) % 256
        slab = pad_wrap_cols(xx[b_][rows]).astype(np.float32)
        in_maps.append({**shared, "x_slab": np.ascontiguousarray(slab)})
    return in_maps


def kernel(x, w1, b1, w2, b2, w3, b3, w4, b4, n_it):
    assert int(n_it) == N_IT
    x = np.asarray(x, np.float32)
    if "nc" not in _CACHE:
        _CACHE["nc"] = build_nc()
    nc = _CACHE["nc"]
    in_maps = _host_inputs(
        x, np.asarray(w1, np.float32), np.asarray(b1, np.float32),
        np.asarray(w2, np.float32), np.asarray(b2, np.float32),
        np.asarray(w3, np.float32), np.asarray(b3, np.float32),
        np.asarray(w4, np.float32), np.asarray(b4, np.float32),
    )
    res = run_bass_kernel_spmd(nc, in_maps, core_ids=list(range(8)))
    out = np.zeros((4, 1, 256, 256), np.float32)
    for c in range(8):
        b_, half = c // 2, c % 2
        out[b_, 0, 128 * half : 128 * half + 128, :] = res.results[c]["out"]
    return out
